# revision 15
# baseline (speedup 1.0000x reference)
"""GCN 2-layer (gcn_norm) SPMD Bass kernel for 8 TRN2 NeuronCores — v5.

Improvements over baseline:
  - tables in bf16, rows padded to 128 ch (256B) -> gathered messages are
    bf16 directly; scatter matmuls run fp8(one-hot) x bf16 at 1 cyc/row.
  - indicators (one-hot scatter matrices) precomputed on host in fp8e4m3
    and DMA-streamed per tile; removes all IS_EQ/ABS/RELU indicator
    builds from DVE/Scalar (~1.2ms of engine time in baseline).
  - dma_gather issued with IDXG indices per instruction (Pool-engine
    SWDGE desc-gen has ~4.2us fixed cost per instruction; fewer+larger
    instructions cut Pool busy time).
  - per-(tile,half) chunk caps = max over cores (less padding).
  - bias broadcasts pre-tiled on host (no Pool partition_broadcast).
  - layer-1 phase B runs tiles 24..48 first so the layer-2 hi bounce
    completes mid-layer; AG2-hi fires between the two halves and overlaps
    the rest of layer 1; layer 2 consumes hi chunks first so it never
    stalls on AG2-lo. Gather packs are pre-issued per half with the
    early-table stream leading (LEAD) so the Pool engine never blocks
    on a collective for a not-yet-needed table half.

Math (per layer): out = dis*(sum_{e->v} dis_src*h_src) + dis^2*h_v + b
with dis = deg^-1/2 (deg incl self-loop), h = x@W.
"""

import numpy as np
import ml_dtypes

N_NODES = 50000
N_EDGES = 800000
IN_CH = 128
HID = 64
OUT = 64
N_CORES = 8
PER_CORE = N_NODES // N_CORES          # 6250
N_TILES = (PER_CORE + 127) // 128      # 49
HALF_LOC = PER_CORE // 2               # 3125
TAB_ROWS = N_CORES * HALF_LOC          # 25000
PADW = 128                             # padded row width (bf16 -> 256B)

IDXG = 1024                            # indices per dma_gather (hard ucode cap)
NQUEUES = 4

_compiled_cache = {}


def _preprocess(edge_index: np.ndarray):
    """Host-side graph preprocessing.

    Returns dis, per-core dict(idx_lo, idx_hi, ind), c_lo[t], c_hi[t].
    """
    src = edge_index[0].astype(np.int64)
    dst = edge_index[1].astype(np.int64)

    deg = np.bincount(dst, minlength=N_NODES).astype(np.float64) + 1.0
    dis = (1.0 / np.sqrt(deg)).astype(np.float32)

    # table mapping: node v -> (half, row)   (same as baseline)
    src_core = src // PER_CORE
    src_r = src % PER_CORE
    half = (src_r >= HALF_LOC).astype(np.int64)
    tab_row = src_core * HALF_LOC + (src_r % HALF_LOC)

    core = dst // PER_CORE
    tile = (dst - core * PER_CORE) // 128
    dloc = dst - core * PER_CORE - tile * 128      # 0..127
    order = np.lexsort((src, half, tile, core))
    row_s = tab_row[order]
    core_s, tile_s, half_s, dloc_s = core[order], tile[order], half[order], \
        dloc[order]

    gid = (core_s * N_TILES + tile_s) * 2 + half_s
    counts = np.bincount(gid, minlength=N_CORES * N_TILES * 2).reshape(
        N_CORES, N_TILES, 2)
    # per-(tile, half) caps: max over cores, rounded up to 128
    caps = counts.max(axis=0)                      # [N_TILES, 2]
    caps = np.maximum(128, ((caps + 127) // 128) * 128)
    c_lo = (caps[:, 0] // 128).astype(np.int64)
    c_hi = (caps[:, 1] // 128).astype(np.int64)
    tot_lo = int(c_lo.sum())
    tot_hi = int(c_hi.sum())
    tot_ch = tot_lo + tot_hi

    starts = np.zeros(N_CORES * N_TILES * 2 + 1, dtype=np.int64)
    np.cumsum(counts.reshape(-1), out=starts[1:])

    # chunk-stream offsets
    lo_off = np.concatenate([[0], np.cumsum(c_lo)])   # [N_TILES+1]
    hi_off = np.concatenate([[0], np.cumsum(c_hi)])

    def wrap(a):
        """[T_chunks*128] idx -> [128, chunks*8] int16 (16-wrap, tiled x8)."""
        n = a.shape[0]
        w = a.reshape(n // 16, 16).T.astype(np.int16)  # [16, n/16]
        return np.tile(w, (8, 1)).copy()

    per_core = []
    for c in range(N_CORES):
        idx_lo = np.zeros(tot_lo * 128, dtype=np.int64)
        idx_hi = np.zeros(tot_hi * 128, dtype=np.int64)
        # fp8 indicator, chunk-major: per tile: lo chunks then hi chunks
        ind = np.zeros((128, tot_ch * 128), dtype=ml_dtypes.float8_e4m3)
        cblock = 0
        for t in range(N_TILES):
            for h, (idxarr, off, ch_t) in enumerate((
                    (idx_lo, lo_off[t], c_lo[t]),
                    (idx_hi, hi_off[t], c_hi[t]))):
                g = (c * N_TILES + t) * 2 + h
                nn = counts[c, t, h]
                s0 = starts[g]
                idxarr[off * 128:off * 128 + nn] = row_s[s0:s0 + nn]
                dl = dloc_s[s0:s0 + nn]
                e_slot = np.arange(nn) % 128
                e_chunk = np.arange(nn) // 128
                ind[e_slot, (cblock + e_chunk) * 128 + dl] = 1.0
                cblock += ch_t
        per_core.append(dict(
            idx_lo=wrap(idx_lo),
            idx_hi=wrap(idx_hi),
            ind=ind,
        ))
    return dis, per_core, tuple(c_lo.tolist()), tuple(c_hi.tolist())


def _build(c_lo, c_hi):
    import concourse.bacc as bacc
    import concourse.mybir as mybir
    import concourse.tile as tile
    from concourse.bass import ds, ts

    f32 = mybir.dt.float32
    bf16 = mybir.dt.bfloat16
    fp8 = mybir.dt.float8e4
    i16 = mybir.dt.int16

    c_t = [a + b for a, b in zip(c_lo, c_hi)]
    tot_lo, tot_hi = sum(c_lo), sum(c_hi)
    tot_ch = tot_lo + tot_hi
    lo_off = [int(v) for v in np.concatenate([[0], np.cumsum(c_lo)])]
    hi_off = [int(v) for v in np.concatenate([[0], np.cumsum(c_hi)])]
    ind_off = [int(v) for v in np.concatenate([[0], np.cumsum(c_t)])]
    max_ct = max(c_t)
    CPG = IDXG // 128

    nc = bacc.Bacc("TRN2", target_bir_lowering=False, debug=False,
                   num_devices=N_CORES, dynamic_dma_scratch_size=65536,
                   num_swdge_queues=NQUEUES)

    # ---- I/O ----
    xT_d = nc.dram_tensor("xT", [IN_CH, PER_CORE], f32, kind="ExternalInput")
    w1_d = nc.dram_tensor("w1", [IN_CH, HID], f32, kind="ExternalInput")
    w2_d = nc.dram_tensor("w2", [HID, OUT], f32, kind="ExternalInput")
    b1_d = nc.dram_tensor("b1", [128, HID], f32, kind="ExternalInput")
    b2_d = nc.dram_tensor("b2", [128, OUT], f32, kind="ExternalInput")
    dis_d = nc.dram_tensor("dis_t", [128, N_TILES], f32, kind="ExternalInput")
    dis2_d = nc.dram_tensor("dis2_t", [128, N_TILES], f32,
                            kind="ExternalInput")
    ixlo_d = nc.dram_tensor("idx_lo", [128, tot_lo * 8], i16,
                            kind="ExternalInput")
    ixhi_d = nc.dram_tensor("idx_hi", [128, tot_hi * 8], i16,
                            kind="ExternalInput")
    ind_d = nc.dram_tensor("ind", [128, tot_ch * 128], fp8,
                           kind="ExternalInput")
    out_d = nc.dram_tensor("out_local", [PER_CORE, OUT], f32,
                           kind="ExternalOutput")

    # internal DRAM: per-layer half bounces + half tables (bf16, 128-wide)
    bnc = {}
    tab = {}
    for layer in (1, 2):
        for st in ("lo", "hi"):
            bnc[layer, st] = nc.dram_tensor(
                f"bounce{layer}{st}", [HALF_LOC, PADW], bf16, kind="Internal")
            tab[layer, st] = nc.dram_tensor(
                f"table{layer}{st}", [TAB_ROWS, PADW], bf16, kind="Internal",
                addr_space="Shared")

    ident_np = np.eye(128, dtype=np.float32)
    ident_d = nc.inline_tensor(ident_np, name="ident128")

    with tile.TileContext(nc) as tc:
        with (
            tc.tile_pool(name="const", bufs=1) as cpool,
            tc.tile_pool(name="state", bufs=1) as spool,
            tc.tile_pool(name="work", bufs=6) as wpool,
            tc.tile_pool(name="glo", bufs=16) as gpool_lo,
            tc.tile_pool(name="ghi", bufs=16) as gpool_hi,
            tc.tile_pool(name="indp", bufs=4) as ipool,
            tc.tile_pool(name="psA", bufs=3, space="PSUM") as psA,
            tc.tile_pool(name="psB", bufs=4, space="PSUM") as psB,
            tc.tile_pool(name="psT", bufs=1, space="PSUM") as psT,
        ):
            # ---- constants / inputs to SBUF ----
            ident_sb = cpool.tile([128, 128], f32, tag="ident")
            nc.sync.dma_start(ident_sb[:], ident_d[:])
            w1_sb = cpool.tile([IN_CH, HID], f32, tag="w1")
            nc.sync.dma_start(w1_sb[:], w1_d[:])
            w2_sb = cpool.tile([HID, OUT], f32, tag="w2")
            nc.sync.dma_start(w2_sb[:], w2_d[:])
            dis_sb = cpool.tile([128, N_TILES], f32, tag="dis")
            nc.sync.dma_start(dis_sb[:], dis_d[:])
            dis2_sb = cpool.tile([128, N_TILES], f32, tag="dis2")
            nc.sync.dma_start(dis2_sb[:], dis2_d[:])
            b1_bc = cpool.tile([128, HID], f32, tag="b1b")
            nc.sync.dma_start(b1_bc[:], b1_d[:])
            b2_bc = cpool.tile([128, OUT], f32, tag="b2b")
            nc.sync.dma_start(b2_bc[:], b2_d[:])
            ixlo_sb = cpool.tile([128, tot_lo * 8], i16, tag="ixlo")
            nc.sync.dma_start(ixlo_sb[:], ixlo_d[:])
            ixhi_sb = cpool.tile([128, tot_hi * 8], i16, tag="ixhi")
            nc.sync.dma_start(ixhi_sb[:], ixhi_d[:])
            xT_sb = cpool.tile([IN_CH, PER_CORE], f32, tag="xT")
            for i in range(4):
                w = (PER_CORE + 3) // 4
                lo = i * w
                hi = min(PER_CORE, lo + w)
                nc.sync.dma_start(xT_sb[:, ds(lo, hi - lo)],
                                  xT_d[:, ds(lo, hi - lo)])

            # per-tile state tiles
            s1_t = [spool.tile([128, HID], f32, tag=f"s1_{t}",
                                name=f"s1_{t}") for t in range(N_TILES)]
            s2_t = [spool.tile([128, OUT], f32, tag=f"s2_{t}",
                                name=f"s2_{t}") for t in range(N_TILES)]
            h1_t = [spool.tile([128, HID], f32, tag=f"h1_{t}",
                                name=f"h1_{t}") for t in range(N_TILES)]
            nc.vector.memset(h1_t[N_TILES - 1][:], 0.0)

            def bounce_store(layer, t, nt, src_tile):
                eng = nc.sync if t % 2 == 0 else nc.scalar
                r0 = t * 128
                r1 = r0 + nt
                if r1 <= HALF_LOC:
                    eng.dma_start(bnc[layer, "lo"][ds(r0, nt), :],
                                  src_tile[:nt, :])
                elif r0 >= HALF_LOC:
                    eng.dma_start(bnc[layer, "hi"][ds(r0 - HALF_LOC, nt), :],
                                  src_tile[:nt, :])
                else:
                    n_a = HALF_LOC - r0
                    eng.dma_start(bnc[layer, "lo"][ds(r0, n_a), :],
                                  src_tile[:n_a, :])
                    eng.dma_start(bnc[layer, "hi"][ds(0, nt - n_a), :],
                                  src_tile[n_a:nt, :])

            def all_gather(layer, s):
                nc.gpsimd.collective_compute(
                    "AllGather", mybir.AluOpType.bypass,
                    replica_groups=[list(range(N_CORES))],
                    ins=[bnc[layer, s][:]], outs=[tab[layer, s][:]])

            def phase_a1(t, nt):
                ps = psA.tile([128, HID], f32, tag="psa")
                nc.tensor.matmul(ps[:nt, :], xT_sb[:, ds(t * 128, nt)],
                                 w1_sb[:], start=True, stop=True)
                hp = wpool.tile([128, PADW], bf16, tag="hp")
                dcol = dis_sb[:nt, t:t + 1]
                nc.scalar.mul(hp[:nt, 0:HID], ps[:nt, :], dcol)
                nc.vector.scalar_tensor_tensor(
                    s1_t[t][:nt, :], ps[:nt, :], dis2_sb[:nt, t:t + 1],
                    b1_bc[:nt, :], mybir.AluOpType.mult, mybir.AluOpType.add)
                bounce_store(1, t, nt, hp)

            def phase_a2(t, nt):
                pt = psT.tile([HID, 128], f32, tag="pst")
                nc.tensor.transpose(pt[:], h1_t[t][:], ident_sb[:])
                hT = wpool.tile([HID, 128], f32, tag="hT")
                nc.scalar.copy(hT[:], pt[:])
                ps = psA.tile([128, OUT], f32, tag="psa")
                nc.tensor.matmul(ps[:], hT[:], w2_sb[:], start=True, stop=True)
                gp = wpool.tile([128, PADW], bf16, tag="gp")
                dcol = dis_sb[:nt, t:t + 1]
                nc.scalar.mul(gp[:nt, 0:OUT], ps[:nt, :], dcol)
                nc.vector.scalar_tensor_tensor(
                    s2_t[t][:nt, :], ps[:nt, :], dis2_sb[:nt, t:t + 1],
                    b2_bc[:nt, :], mybir.AluOpType.mult, mybir.AluOpType.add)
                bounce_store(2, t, nt, gp)

            CPGl = CPG
            LEAD = 16

            def phase_b(layer, ch, tile_order, early, tile_done=None,
                        mid_cb=None, mid_after_tiles=None):
                qctr = [0]
                pools = {"lo": gpool_lo, "hi": gpool_hi}
                streams = {
                    "lo": (tab[layer, "lo"], ixlo_sb, tot_lo, lo_off, c_lo),
                    "hi": (tab[layer, "hi"], ixhi_sb, tot_hi, hi_off, c_hi),
                }
                late = "hi" if early == "lo" else "lo"
                halves = ([tile_order[:mid_after_tiles],
                           tile_order[mid_after_tiles:]]
                          if mid_after_tiles else [tile_order])
                gtiles = {"lo": {}, "hi": {}}
                cmaps = {"lo": {}, "hi": {}}

                def pack_windows(stream, tiles):
                    """segment-aligned (c0, n_ch) windows, consumption
                    order; fills cmaps[stream]."""
                    _, _, total, off, c_arr = streams[stream]
                    runs = []
                    for t in tiles:
                        c0, c1 = off[t], off[t] + c_arr[t]
                        if runs and runs[-1][1] == c0:
                            runs[-1][1] = c1
                        else:
                            runs.append([c0, c1])
                    wins = []
                    for r0, r1 in runs:
                        c = r0
                        while c < r1:
                            n = min(CPGl, r1 - c)
                            wins.append((c, n))
                            for j in range(n):
                                cmaps[stream][c + j] = (c, j)
                            c += n
                    return wins

                def issue_packs(tiles):
                    wins_e = [(early, w) for w in pack_windows(early, tiles)]
                    wins_l = [(late, w) for w in pack_windows(late, tiles)]
                    issue = list(wins_e[:LEAD])
                    ei, li = LEAD, 0
                    while ei < len(wins_e) or li < len(wins_l):
                        if li < len(wins_l):
                            issue.append(wins_l[li]); li += 1
                        if ei < len(wins_e):
                            issue.append(wins_e[ei]); ei += 1
                    for s, (c0, n_ch) in issue:
                        table_d, ix_sb, _, _, _ = streams[s]
                        tl = pools[s].tile([128, CPGl, PADW], bf16,
                                           tag="g" + s,
                                           name=f"g{s}{layer}_{c0}")
                        nc.gpsimd.dma_gather(
                            out_ap=tl[:, 0:n_ch, :],
                            in_ap=table_d[:],
                            idxs_ap=ix_sb[:, ds(c0 * 8, n_ch * 8)],
                            num_idxs=n_ch * 128,
                            num_idxs_reg=n_ch * 128,
                            elem_size=PADW,
                            queue_num=qctr[0] % NQUEUES,
                        )
                        qctr[0] += 1
                        gtiles[s][c0] = tl

                def consume_tiles(tiles):
                    for t in tiles:
                        nt = min(128, PER_CORE - t * 128)
                        ind_sb = ipool.tile([128, max_ct * 128], fp8,
                                            tag="ind", name=f"ind{layer}_{t}")
                        nc.scalar.dma_start(
                            ind_sb[:, 0:c_t[t] * 128],
                            ind_d[:, ds(ind_off[t] * 128, c_t[t] * 128)])
                        ps = psB.tile([128, ch], f32, tag="psb")
                        order = ("lo", "hi") if early == "lo" else                             ("hi", "lo")
                        first = True
                        n_tot = c_lo[t] + c_hi[t]
                        done = 0
                        for s in order:
                            _, _, _, off, c_arr = streams[s]
                            kbase = 0 if s == "lo" else c_lo[t]
                            for j in range(c_arr[t]):
                                c0, slot = cmaps[s][off[t] + j]
                                tl = gtiles[s][c0]
                                done += 1
                                nc.tensor.matmul(
                                    ps[:], ind_sb[:, ts(kbase + j, 128)],
                                    tl[:, slot, 0:ch],
                                    start=first, stop=(done == n_tot))
                                first = False
                        dcol = dis_sb[:nt, t:t + 1]
                        if layer == 1:
                            nc.vector.scalar_tensor_tensor(
                                h1_t[t][:nt, :], ps[:nt, :], dcol,
                                s1_t[t][:nt, :],
                                mybir.AluOpType.mult, mybir.AluOpType.add)
                            nc.scalar.activation(
                                h1_t[t][:nt, :], h1_t[t][:nt, :],
                                mybir.ActivationFunctionType.Relu)
                        else:
                            ot = wpool.tile([128, OUT], f32, tag="ot")
                            nc.vector.scalar_tensor_tensor(
                                ot[:nt, :], ps[:nt, :], dcol,
                                s2_t[t][:nt, :],
                                mybir.AluOpType.mult, mybir.AluOpType.add)
                            nc.sync.dma_start(out_d[ds(t * 128, nt), :],
                                              ot[:nt, :])
                        if tile_done is not None:
                            tile_done(t, nt)

                for hidx, tiles in enumerate(halves):
                    issue_packs(tiles)
                    consume_tiles(tiles)
                    if mid_cb is not None and hidx == 0 and len(halves) > 1:
                        mid_cb()

            # ---------- layer 1 ----------
            for t in range(N_TILES):
                phase_a1(t, min(128, PER_CORE - t * 128))
                if t == 24:
                    all_gather(1, "lo")
            all_gather(1, "hi")

            # B1 processes tiles 24..48 first (tile 24 writes the tail of
            # bounce-lo AND the head of bounce-hi) so bounce2-hi completes
            # early; AG2-hi fires mid-stream, AG2-lo after the last tile.
            order_b1 = list(range(24, N_TILES)) + list(range(0, 24))

            def l1_done(t, nt):
                phase_a2(t, nt)
                if t == 23:           # processed last
                    all_gather(2, "lo")

            phase_b(1, HID, order_b1, "lo", tile_done=l1_done,
                    mid_cb=lambda: all_gather(2, "hi"),
                    mid_after_tiles=N_TILES - 24)

            # ---------- layer 2 ----------
            phase_b(2, OUT, list(range(N_TILES)), "hi")

    nc.compile()
    return nc


def _make_in_maps(x, W1, b1, W2, b2, dis, per_core):
    in_maps = []
    dis_pad = np.zeros(N_TILES * 128, dtype=np.float32)
    for c in range(N_CORES):
        dis_c = dis_pad.copy()
        dis_c[:PER_CORE] = dis[c * PER_CORE:(c + 1) * PER_CORE]
        dis_t = np.ascontiguousarray(dis_c.reshape(N_TILES, 128).T)
        in_maps.append({
            "xT": np.ascontiguousarray(x[c * PER_CORE:(c + 1) * PER_CORE].T),
            "w1": np.ascontiguousarray(W1),
            "w2": np.ascontiguousarray(W2),
            "b1": np.ascontiguousarray(np.tile(b1.reshape(1, -1), (128, 1))),
            "b2": np.ascontiguousarray(np.tile(b2.reshape(1, -1), (128, 1))),
            "dis_t": dis_t,
            "dis2_t": np.ascontiguousarray(dis_t * dis_t),
            "idx_lo": per_core[c]["idx_lo"],
            "idx_hi": per_core[c]["idx_hi"],
            "ind": per_core[c]["ind"],
        })
    return in_maps


def run(x, edge_index, W1, b1, W2, b2, trace=False):
    from concourse.bass_utils import run_bass_kernel_spmd

    x = np.asarray(x, dtype=np.float32)
    edge_index = np.asarray(edge_index)
    W1 = np.asarray(W1, dtype=np.float32)
    b1 = np.asarray(b1, dtype=np.float32)
    W2 = np.asarray(W2, dtype=np.float32)
    b2 = np.asarray(b2, dtype=np.float32)

    dis, per_core, c_lo, c_hi = _preprocess(edge_index)
    key = (c_lo, c_hi)
    if key not in _compiled_cache:
        _compiled_cache[key] = _build(c_lo, c_hi)
    nc = _compiled_cache[key]
    in_maps = _make_in_maps(x, W1, b1, W2, b2, dis, per_core)
    res = run_bass_kernel_spmd(nc, in_maps, core_ids=list(range(N_CORES)),
                               trace=trace)
    out = np.concatenate([res.results[c]["out_local"] for c in range(N_CORES)],
                         axis=0)
    return out, res


def kernel(x, edge_index, W1, b1, W2, b2):
    out, _ = run(x, edge_index, W1, b1, W2, b2, trace=False)
    return out
